# revision 9
# baseline (speedup 1.0000x reference)
"""Trainium2 Bass kernel for nn_DiseaseKnowledgeModule.

Reference computation (per batch b):
    z_pooled = mean(z_fused[b], axis=S)                      # [D]
    scores   = z_pooled @ M[n,s,:] / sqrt(D)                 # [14, 2]
    alpha    = softmax(scores, axis=-1)
    mlc[b]   = alpha[:, 1]                                   # sigmoid(s1-s0)
    ah       = (mlc[b] > 0.2)                                # {0,1}
    R        = ah @ M[:, 1, :]                               # [D]
    z_out[b] = z_fused[b] + R                                # broadcast over S

Sharding: data-parallel over batch, 2 batches per core on 8 cores; M
replicated.  Memory-bound: per core 32 MB in + 32 MB out.  Each batch
(16 MB) is kept SBUF-resident between the pooling pass and the
broadcast-add so z is read exactly once.

Layout per core: z[b] viewed as chunks [128(S) x 4096] (2 MiB DMAs, S on
partitions, 4x1024 on free).  Pooling = PE matmul with a ones[128,1]
stationary vector (float32r: 1 cycle/row) accumulating into PSUM.  The
tiny softmax/threshold chain runs on PE/DVE/ACT; R is broadcast to 128
partitions with a ones[1,128] fp32 matmul; DVE adds R in place; chunks
stream back out.  A 10-slot ring lets the next batch's loads overlap the
previous batch's drain.
"""

import numpy as np

B, S, D = 16, 4096, 1024
ND, NS = 14, 2
THRESH = 0.2
NCORES = 8
BPC = B // NCORES          # batches per core
NCHUNK = 8                 # chunks per batch
CHUNK_Q = 4                # 128-row subtiles per chunk
CHUNK_F = CHUNK_Q * D      # free elements per chunk (4096)
NSLOT = 9                  # resident ring slots (9 x 2 MiB = 18 MB SBUF)
SCALE = 1.0 / (S * float(np.sqrt(D)))  # fold mean and 1/sqrt(D): 2**-17

_CACHE = {}
LAST_RESULTS = None


def _build_nc():
    import concourse.bacc as bacc
    import concourse.mybir as mybir
    import concourse.tile as tile
    from concourse.masks import make_identity

    f32 = mybir.dt.float32
    bf16 = mybir.dt.bfloat16

    nc = bacc.Bacc("TRN2", target_bir_lowering=False)
    z = nc.dram_tensor("z", [BPC, S, D], f32, kind="ExternalInput")
    m = nc.dram_tensor("m", [ND, NS, D], f32, kind="ExternalInput")
    z_out = nc.dram_tensor("z_out", [BPC, S, D], f32, kind="ExternalOutput")
    mlc_out = nc.dram_tensor("mlc", [BPC, ND], f32, kind="ExternalOutput")

    with tile.TileContext(nc) as tc:
        with (
            tc.tile_pool(name="const", bufs=1) as const,
            tc.tile_pool(name="res", bufs=1) as respool,
            tc.tile_pool(name="small", bufs=2) as small,
            tc.tile_pool(name="bfp", bufs=2) as bfp,
            tc.tile_pool(name="rbc", bufs=2) as rbcpool,
            tc.tile_pool(name="ppsum", bufs=1, space="PSUM") as ppsum,
            tc.tile_pool(name="bpsum", bufs=1, space="PSUM") as bpsum,
            tc.tile_pool(name="spsum", bufs=2, space="PSUM") as spsum,
        ):
            # ---- constants ----
            ones_k = const.tile([128, 1], bf16, tag="ones_k")
            nc.vector.memset(ones_k[:], 1.0)
            one_1 = const.tile([1, 1], f32, tag="one_1")
            nc.vector.memset(one_1[:], 1.0)
            ones_r = const.tile([1, 128], f32, tag="ones_r")
            nc.vector.memset(ones_r[:], 1.0)
            ident = const.tile([28, 28], f32, tag="ident")
            make_identity(nc, ident[:])

            m_nat = const.tile([2 * ND, D], f32, tag="m_nat")
            nc.sync.dma_start(out=m_nat[:], in_=m[:].rearrange("n s d -> (n s) d"))
            m_pres = const.tile([ND, D], f32, tag="m_pres")
            nc.sync.dma_start(out=m_pres[:], in_=m[:, 1, :])

            # M^T chunks: m_t[:, 28c:28c+28][p, ns] = M_flat[ns, 128c+p]
            m_t = const.tile([128, 8 * 2 * ND], f32, tag="m_t")
            for c in range(8):
                tr_ps = spsum.tile([128, 2 * ND], f32, tag="setup")
                nc.tensor.transpose(
                    tr_ps[:], m_nat[:, c * 128 : (c + 1) * 128], ident[:]
                )
                nc.vector.tensor_copy(
                    out=m_t[:, c * 2 * ND : (c + 1) * 2 * ND], in_=tr_ps[:]
                )

            # ---- resident ring ----
            res = respool.tile([128, NSLOT * CHUNK_F], f32, tag="res")

            z_v = z[:].rearrange("b (i q p) d -> b i p q d", p=128, q=CHUNK_Q)
            zo_v = z_out[:].rearrange("b (i q p) d -> b i p q d", p=128, q=CHUNK_Q)

            for b in range(BPC):
                # ---- phase A: stream in + pooling matmuls ----
                pacc = ppsum.tile([1, 1024], f32, tag="pacc")
                for i in range(NCHUNK):
                    slot = (b * NCHUNK + i) % NSLOT
                    sl = res[:, slot * CHUNK_F : (slot + 1) * CHUNK_F]
                    nc.sync.dma_start(
                        out=sl.rearrange("p (q d) -> p q d", q=CHUNK_Q),
                        in_=z_v[b, i],
                    )
                    # bf16 shadow copy (ScalarE, otherwise idle) so the
                    # pooling matmul streams at 1 cycle/row.  Only the
                    # pooled->softmax path sees bf16; z_out stays exact.
                    bft = bfp.tile([128, CHUNK_F], bf16, tag="bfs")
                    nc.scalar.activation(
                        out=bft[:],
                        in_=sl,
                        func=mybir.ActivationFunctionType.Copy,
                    )
                    for j in range(8):  # 512-wide columns; d-half h = j % 2
                        h = j % 2
                        nc.tensor.matmul(
                            pacc[:, h * 512 : (h + 1) * 512],
                            ones_k[:],
                            bft[:, j * 512 : (j + 1) * 512],
                            start=(i == 0 and j < 2),
                            stop=(i == NCHUNK - 1 and j >= 6),
                        )

                # ---- phase B: scores -> sigmoid -> threshold -> R ----
                pooled = small.tile([1, 1024], f32, tag="pooled")
                nc.scalar.activation(
                    out=pooled[:],
                    in_=pacc[:],
                    func=mybir.ActivationFunctionType.Copy,
                    scale=SCALE,
                )
                # pooled^T into [128, 8] via K=1 matmuls
                pt_ps = bpsum.tile([128, 8], f32, tag="small")
                for c in range(8):
                    nc.tensor.matmul(
                        pt_ps[:, c : c + 1],
                        pooled[:, c * 128 : (c + 1) * 128],
                        one_1[:],
                        start=True,
                        stop=True,
                    )
                pooled_t = small.tile([128, 8], f32, tag="pooled_t")
                nc.vector.tensor_copy(out=pooled_t[:], in_=pt_ps[:])

                sc_ps = bpsum.tile([1, 2 * ND], f32, tag="small")
                for c in range(8):
                    nc.tensor.matmul(
                        sc_ps[:],
                        pooled_t[:, c : c + 1],
                        m_t[:, c * 2 * ND : (c + 1) * 2 * ND],
                        start=(c == 0),
                        stop=(c == 7),
                    )

                sc_sb = small.tile([1, 2 * ND], f32, tag="sc_sb")
                nc.vector.tensor_copy(out=sc_sb[:], in_=sc_ps[:])
                sc_v = sc_sb[:].rearrange("p (n s) -> p n s", s=2)
                diff = small.tile([1, ND], f32, tag="diff")
                nc.vector.tensor_sub(diff[:], sc_v[:, :, 1], sc_v[:, :, 0])
                mlc_sb = small.tile([1, ND], f32, tag="mlc_sb")
                nc.scalar.activation(
                    out=mlc_sb[:],
                    in_=diff[:],
                    func=mybir.ActivationFunctionType.Sigmoid,
                )
                nc.sync.dma_start(out=mlc_out[b : b + 1, :], in_=mlc_sb[:])

                ah = small.tile([1, ND], f32, tag="ah")
                nc.vector.tensor_scalar(
                    out=ah[:],
                    in0=mlc_sb[:],
                    scalar1=THRESH,
                    scalar2=None,
                    op0=mybir.AluOpType.is_gt,
                )
                ah_ps = bpsum.tile([ND, 1], f32, tag="small")
                nc.tensor.matmul(ah_ps[:], ah[:], one_1[:], start=True, stop=True)
                ah_t = small.tile([ND, 1], f32, tag="ah_t")
                nc.vector.tensor_copy(out=ah_t[:], in_=ah_ps[:])

                r_ps = bpsum.tile([1, 1024], f32, tag="r")
                for hh in range(2):
                    nc.tensor.matmul(
                        r_ps[:, hh * 512 : (hh + 1) * 512],
                        ah_t[:],
                        m_pres[:, hh * 512 : (hh + 1) * 512],
                        start=True,
                        stop=True,
                    )
                r_sb = small.tile([1, 1024], f32, tag="r_sb")
                nc.vector.tensor_copy(out=r_sb[:], in_=r_ps[:])

                # broadcast R to 128 partitions (exact fp32 ones-matmul)
                r_bc = rbcpool.tile([128, 1024], f32, tag="r_bc")
                for hh in range(2):
                    bc_ps = bpsum.tile([128, 512], f32, tag="r")
                    nc.tensor.matmul(
                        bc_ps[:],
                        ones_r[:],
                        r_sb[:, hh * 512 : (hh + 1) * 512],
                        start=True,
                        stop=True,
                    )
                    nc.vector.tensor_copy(
                        out=r_bc[:, hh * 512 : (hh + 1) * 512], in_=bc_ps[:]
                    )

                # ---- phase C: add R in place, stream out ----
                for i in range(NCHUNK):
                    slot = (b * NCHUNK + i) % NSLOT
                    sl = res[:, slot * CHUNK_F : (slot + 1) * CHUNK_F]
                    for q in range(CHUNK_Q):
                        nc.vector.tensor_add(
                            sl[:, q * D : (q + 1) * D],
                            sl[:, q * D : (q + 1) * D],
                            r_bc[:],
                        )
                    nc.sync.dma_start(
                        out=zo_v[b, i],
                        in_=sl.rearrange("p (q d) -> p q d", q=CHUNK_Q),
                    )

    nc.finalize()
    return nc


def _get_nc():
    if "nc" not in _CACHE:
        _CACHE["nc"] = _build_nc()
    return _CACHE["nc"]


def kernel(z_fused, M):
    import os

    from concourse.bass_utils import run_bass_kernel_spmd

    global LAST_RESULTS
    nc = _get_nc()
    z_fused = np.ascontiguousarray(z_fused, dtype=np.float32)
    M = np.ascontiguousarray(M, dtype=np.float32)
    in_maps = [
        {"z": z_fused[c * BPC : (c + 1) * BPC], "m": M} for c in range(NCORES)
    ]
    kwargs = {}
    if os.environ.get("KERNEL_TRACE"):
        kwargs["trace"] = True
        if os.environ.get("KERNEL_TMPDIR"):
            kwargs["tmpdir"] = os.environ["KERNEL_TMPDIR"]
    res = run_bass_kernel_spmd(nc, in_maps, list(range(NCORES)), **kwargs)
    LAST_RESULTS = res
    z_out = np.concatenate(
        [res.results[c]["z_out"] for c in range(NCORES)], axis=0
    )
    mlc = np.concatenate([res.results[c]["mlc"] for c in range(NCORES)], axis=0)
    return z_out, mlc
